# revision 4
# baseline (speedup 1.0000x reference)
"""CQT (constant-Q transform) kernel for Trainium2, 8 NeuronCores.

Math: out[b, c, t] = sum_l W[c, l] * x_pad[b, t*HOP + l]   (strided conv,
HOP=512, L=11339 taps, C=168 channels = 84 bins x re/im), then reshaped to
(B, 2, n_bins, T_out).

Strategy:
  - Data-parallel: shard B=32 across 8 cores (4 batches/core), weights
    replicated.
  - The conv is decomposed into 128-tap blocks: block i covers taps
    [128*i, 128*i+128).  For each block, out[c, t] += Wblk_i[:, c].T @
    X_i[:, t] is a matmul with K=128 on partitions.  The moving operand for
    block i=(4j+k) at output tile [t0, t0+NT) is a contiguous column slice
    of a host-pre-transposed view of x:  xt[r, k, u] = x_pad[512u+128k+r].
  - CQT kernels are ragged (bin k has ~11339*2^(-k/12) taps, centered), so
    the active channels of each block form a prefix; blocks run with
    M = (last nonzero channel + 1) only.  PSUM accumulates all blocks per
    output tile (bank0 = channels 0:128, bank1 = 128:C), fp32r matmuls
    (1 cycle/row at N>=256, FP22 multiply / FP32 accumulate).
"""

import numpy as np

HOP = 512
N_CORES = 8

_prog_cache: dict = {}


def _host_prep(x, kernels):
    x = np.ascontiguousarray(np.asarray(x, dtype=np.float32))
    kernels = np.ascontiguousarray(np.asarray(kernels, dtype=np.float32))
    B, T = x.shape
    nbins, two, Lmax = kernels.shape
    assert two == 2
    C = 2 * nbins
    pad = Lmax // 2
    T_out = (T + 2 * pad - Lmax) // HOP + 1

    # ---- weights: pad taps to 128 multiple, find ragged active prefixes ----
    nblk_full = -(-Lmax // 128)
    Wp = np.zeros((C, nblk_full * 128), dtype=np.float32)
    Wp[:, :Lmax] = kernels.reshape(C, Lmax)
    nz = (Wp.reshape(C, nblk_full, 128) != 0.0).any(axis=2)  # [C, nblk]
    Ms, keep = [], []
    for i in range(nblk_full):
        idx = np.where(nz[:, i])[0]
        if len(idx):
            keep.append(i)
            Ms.append(int(idx[-1]) + 1)
    keep = np.asarray(keep, dtype=np.int64)
    # SBUF weight layout: wt[r, pos*C + c] = Wp[c, 128*keep[pos] + r]
    wt = np.ascontiguousarray(
        Wp.reshape(C, nblk_full, 128)[:, keep, :].transpose(2, 1, 0).reshape(128, -1)
    )

    # ---- x: pad and pre-transpose to [128, 4, U] per batch ----
    j_max = int(keep.max()) // 4
    U = T_out + j_max
    xpad_len = 512 * U
    assert xpad_len >= pad + T, (xpad_len, pad + T)
    xp = np.zeros((B, xpad_len), dtype=np.float32)
    xp[:, pad:pad + T] = x
    # xt[b, r, k*U + u] = xp[b, 512u + 128k + r]
    xt = np.ascontiguousarray(
        xp.reshape(B, U, 4, 128).transpose(0, 3, 2, 1).reshape(B, 128, 4 * U)
    )
    return xt, wt, keep.tolist(), Ms, C, U, T_out, nbins


def _build_program(b_per, C, U, T_out, keep, Ms):
    import concourse.mybir as mybir
    import concourse.tile as tile
    from concourse import bacc

    f32 = mybir.dt.float32
    f32r = mybir.dt.float32r
    nblk = len(keep)
    mb_max = max(max(Ms) - 128, 0)
    nts = [512] * (T_out // 512) + ([T_out % 512] if T_out % 512 else [])
    order = sorted(range(nblk), key=lambda p: -Ms[p])
    a_ps = [p for p in order if Ms[p] > 0]
    b_ps = [p for p in order if Ms[p] > 128]

    nc = bacc.Bacc(
        "TRN2",
        target_bir_lowering=False,
        debug=False,
        enable_asserts=True,
        num_devices=N_CORES,
    )
    xt_d = nc.dram_tensor("xt", [b_per, 128, 4 * U], f32r, kind="ExternalInput").ap()
    wt_d = nc.dram_tensor("wt", [128, nblk * C], f32r, kind="ExternalInput").ap()
    out_d = nc.dram_tensor("out", [b_per, C, T_out], f32, kind="ExternalOutput").ap()

    with tile.TileContext(nc) as tc:
        with (
            tc.tile_pool(name="wpool", bufs=1) as wpool,
            tc.tile_pool(name="xpool", bufs=2) as xpool,
            tc.tile_pool(name="evpool", bufs=3) as evpool,
            tc.tile_pool(name="pspool", bufs=2, space="PSUM") as pspool,
        ):
            wsb = wpool.tile([128, nblk * C], f32r)
            nc.sync.dma_start(out=wsb[:], in_=wt_d[:])
            for b in range(b_per):
                xb = xpool.tile([128, 4 * U], f32r, tag="xb")
                nc.sync.dma_start(out=xb[:], in_=xt_d[b])
                t0 = 0
                for nt in nts:
                    pa = pspool.tile([128, 512], f32, tag="pa")
                    if mb_max:
                        pb = pspool.tile([128, 512], f32, tag="pb")
                    for pos, p in enumerate(a_ps):
                        m = Ms[p]
                        j, k = divmod(keep[p], 4)
                        rhs = xb[:, k * U + t0 + j: k * U + t0 + j + nt]
                        ma = min(m, 128)
                        nc.tensor.matmul(
                            pa[:ma, :nt],
                            lhsT=wsb[:, p * C: p * C + ma],
                            rhs=rhs,
                            start=(pos == 0),
                            stop=(pos == len(a_ps) - 1),
                        )
                        if m > 128:
                            nc.tensor.matmul(
                                pb[:m - 128, :nt],
                                lhsT=wsb[:, p * C + 128: p * C + m],
                                rhs=rhs,
                                start=(p == b_ps[0]),
                                stop=(p == b_ps[-1]),
                            )
                    ma_all = min(max(Ms), 128)
                    eva = evpool.tile([128, 512], f32, tag="eva")
                    nc.vector.tensor_copy(eva[:ma_all, :nt], pa[:ma_all, :nt])
                    nc.sync.dma_start(
                        out=out_d[b, 0:ma_all, t0:t0 + nt], in_=eva[:ma_all, :nt]
                    )
                    if mb_max:
                        evb = evpool.tile([128, 512], f32, tag="evb")
                        nc.vector.tensor_copy(evb[:mb_max, :nt], pb[:mb_max, :nt])
                        nc.sync.dma_start(
                            out=out_d[b, 128:128 + mb_max, t0:t0 + nt],
                            in_=evb[:mb_max, :nt],
                        )
                    t0 += nt
    nc.compile()
    return nc


def kernel(x, kernels):
    from concourse.bass_utils import run_bass_kernel_spmd

    xt, wt, keep, Ms, C, U, T_out, nbins = _host_prep(x, kernels)
    B = xt.shape[0]
    assert B % N_CORES == 0
    b_per = B // N_CORES

    key = (b_per, C, U, T_out, tuple(keep), tuple(Ms))
    if key not in _prog_cache:
        _prog_cache[key] = _build_program(b_per, C, U, T_out, keep, Ms)
    nc = _prog_cache[key]

    in_maps = [
        {"xt": xt[c * b_per:(c + 1) * b_per], "wt": wt} for c in range(N_CORES)
    ]
    res = run_bass_kernel_spmd(nc, in_maps, list(range(N_CORES)))
    parts = [res.results[c]["out"] for c in range(N_CORES)]
    out = np.concatenate(parts, axis=0)  # (B, C, T_out)
    return np.ascontiguousarray(
        out.reshape(B, nbins, 2, T_out).transpose(0, 2, 1, 3)
    )


# revision 10
# speedup vs baseline: 1.0685x; 1.0685x over previous
"""CQT (constant-Q transform) kernel for Trainium2, 8 NeuronCores.

Math: out[b, c, t] = sum_l W[c, l] * x_pad[b, t*HOP + l]   (strided conv,
HOP=512, L=11339 taps, C=168 channels = 84 bins x re/im), then reshaped to
(B, 2, n_bins, T_out).

Strategy:
  - Data-parallel: shard B=32 across 8 cores (4 batches/core), weights
    replicated.
  - The conv is decomposed into 128-tap blocks: block i covers taps
    [128*i, 128*i+128).  For each block, out[c, t] += Wblk_i[:, c].T @
    X_i[:, t] is a matmul with K=128 on partitions.  The moving operand for
    block i=(4j+k) at output tile [t0, t0+NT) is a contiguous column slice
    of a host-pre-transposed view of x:  xt[r, k, u] = x_pad[512u+128k+r].
  - CQT kernels are ragged (bin k has ~11339*2^(-k/12) taps, centered), so
    the active channels of each block form a prefix; blocks run with
    M = (last nonzero channel + 1) only.  PSUM accumulates all blocks per
    output tile (bank0 = channels 0:128, bank1 = 128:C), fp32r matmuls
    (1 cycle/row at N>=256, FP22 multiply / FP32 accumulate).
"""

import numpy as np

HOP = 512
N_CORES = 8

_prog_cache: dict = {}


def _host_prep(x, kernels):
    x = np.ascontiguousarray(np.asarray(x, dtype=np.float32))
    kernels = np.ascontiguousarray(np.asarray(kernels, dtype=np.float32))
    B, T = x.shape
    nbins, two, Lmax = kernels.shape
    assert two == 2
    C = 2 * nbins
    pad = Lmax // 2
    T_out = (T + 2 * pad - Lmax) // HOP + 1

    # ---- weights: pad taps to 128 multiple, find ragged active prefixes ----
    nblk_full = -(-Lmax // 128)
    Wp = np.zeros((C, nblk_full * 128), dtype=np.float32)
    Wp[:, :Lmax] = kernels.reshape(C, Lmax)
    nz = (Wp.reshape(C, nblk_full, 128) != 0.0).any(axis=2)  # [C, nblk]
    Ms, keep = [], []
    for i in range(nblk_full):
        idx = np.where(nz[:, i])[0]
        if len(idx):
            keep.append(i)
            Ms.append(int(idx[-1]) + 1)
    keep = np.asarray(keep, dtype=np.int64)
    Ms = np.asarray(Ms, dtype=np.int64)
    # order blocks by descending active-channel count: the first matmul per
    # PSUM bank then covers the bank's maximal partition range (required for
    # the start=True zero-region semantics), and the weight DMA can be
    # chunked in exactly the order the matmuls consume it.
    order = np.argsort(-Ms, kind="stable")
    keep = keep[order]
    Ms = Ms[order]
    # SBUF weight layout: wt[r, pos*C + c] = Wp[c, 128*keep[pos] + r]
    wt = np.ascontiguousarray(
        Wp.reshape(C, nblk_full, 128)[:, keep, :].transpose(2, 1, 0).reshape(128, -1)
    )
    keep = keep.tolist()
    Ms = Ms.tolist()

    # ---- x: pad and pre-transpose to [128, 4, U] per batch ----
    j_max = int(max(keep)) // 4
    U = T_out + j_max
    xpad_len = 512 * U
    assert xpad_len >= pad + T, (xpad_len, pad + T)
    xp = np.zeros((B, xpad_len), dtype=np.float32)
    xp[:, pad:pad + T] = x
    # xt[b, r, k*U + u] = xp[b, 512u + 128k + r]
    xt = np.ascontiguousarray(
        xp.reshape(B, U, 4, 128).transpose(0, 3, 2, 1).reshape(B, 128, 4 * U)
    )
    return xt, wt, keep, Ms, C, U, T_out, nbins


def _build_program(b_per, C, U, T_out, keep, Ms):
    import concourse.mybir as mybir
    import concourse.tile as tile
    from concourse import bacc

    f32 = mybir.dt.float32
    f32r = mybir.dt.float32r
    nblk = len(keep)
    mb_max = max(max(Ms) - 128, 0)
    nts = [512] * (T_out // 512) + ([T_out % 512] if T_out % 512 else [])
    # blocks already ordered by descending M in host prep
    a_ps = list(range(nblk))
    b_ps = [p for p in a_ps if Ms[p] > 128]
    j_max = max(keep) // 4
    # weight DMA chunks in matmul consumption order
    WCHUNK = 16
    w_chunks = [
        (p0, min(p0 + WCHUNK, nblk)) for p0 in range(0, nblk, WCHUNK)
    ]
    # x DMA chunks: one per t-tile window (u-ranges, exclusive ends)
    x_stops = []
    t0 = 0
    for nt in nts:
        x_stops.append(min(t0 + nt + j_max + 1, U))
        t0 += nt
    x_stops[-1] = U
    x_chunks = []
    u0 = 0
    for u1 in x_stops:
        if u1 > u0:
            x_chunks.append((u0, u1))
            u0 = u1

    nc = bacc.Bacc(
        "TRN2",
        target_bir_lowering=False,
        debug=False,
        enable_asserts=True,
        num_devices=N_CORES,
    )
    xt_d = nc.dram_tensor("xt", [b_per, 128, 4 * U], f32r, kind="ExternalInput").ap()
    wt_d = nc.dram_tensor("wt", [128, nblk * C], f32r, kind="ExternalInput").ap()
    out_d = nc.dram_tensor("out", [b_per, C, T_out], f32, kind="ExternalOutput").ap()

    with tile.TileContext(nc) as tc:
        with (
            tc.tile_pool(name="wpool", bufs=1) as wpool,
            tc.tile_pool(name="xpool", bufs=2) as xpool,
            tc.tile_pool(name="evpool", bufs=3) as evpool,
            tc.tile_pool(name="pspool", bufs=2, space="PSUM") as pspool,
        ):
            wsb = wpool.tile([128, nblk * C], f32r)

            def dma_x_chunk(xb_tile, b, u0, u1):
                # 3D AP: all 4 k-planes, u in [u0, u1)
                src = xt_d[b].rearrange("r (k u) -> r k u", k=4)[:, :, u0:u1]
                dst = xb_tile.rearrange("r (k u) -> r k u", k=4)[:, :, u0:u1]
                nc.sync.dma_start(out=dst, in_=src)

            # interleave first batch's x chunks with the weight chunks (both
            # in consumption order) so the first sweep starts after ~2.5MB of
            # DMA instead of ~10MB
            xb0 = xpool.tile([128, 4 * U], f32r, tag="xb", name="xb0")
            emits = []
            for i in range(max(len(x_chunks), len(w_chunks))):
                if i < len(x_chunks):
                    emits.append(("x", x_chunks[i]))
                if i < len(w_chunks):
                    emits.append(("w", w_chunks[i]))
            for kind, (a0, a1) in emits:
                if kind == "x":
                    dma_x_chunk(xb0, 0, a0, a1)
                else:
                    nc.sync.dma_start(
                        out=wsb[:, a0 * C: a1 * C], in_=wt_d[:, a0 * C: a1 * C]
                    )

            for b in range(b_per):
                if b == 0:
                    xb = xb0
                else:
                    xb = xpool.tile([128, 4 * U], f32r, tag="xb", name=f"xb{b}")
                    nc.sync.dma_start(out=xb[:], in_=xt_d[b])
                t0 = 0
                for nt in nts:
                    pa = pspool.tile([128, 512], f32, tag="pa")
                    if mb_max:
                        pb = pspool.tile([128, 512], f32, tag="pb")
                    for pos, p in enumerate(a_ps):
                        m = Ms[p]
                        j, k = divmod(keep[p], 4)
                        rhs = xb[:, k * U + t0 + j: k * U + t0 + j + nt]
                        ma = min(m, 128)
                        nc.tensor.matmul(
                            pa[:ma, :nt],
                            lhsT=wsb[:, p * C: p * C + ma],
                            rhs=rhs,
                            start=(pos == 0),
                            stop=(pos == len(a_ps) - 1),
                        )
                        if m > 128:
                            nc.tensor.matmul(
                                pb[:m - 128, :nt],
                                lhsT=wsb[:, p * C + 128: p * C + m],
                                rhs=rhs,
                                start=(p == b_ps[0]),
                                stop=(p == b_ps[-1]),
                            )
                    ma_all = min(max(Ms), 128)
                    eva = evpool.tile([128, 512], f32, tag="eva")
                    nc.vector.tensor_copy(eva[:ma_all, :nt], pa[:ma_all, :nt])
                    nc.sync.dma_start(
                        out=out_d[b, 0:ma_all, t0:t0 + nt], in_=eva[:ma_all, :nt]
                    )
                    if mb_max:
                        evb = evpool.tile([128, 512], f32, tag="evb")
                        nc.vector.tensor_copy(evb[:mb_max, :nt], pb[:mb_max, :nt])
                        nc.sync.dma_start(
                            out=out_d[b, 128:128 + mb_max, t0:t0 + nt],
                            in_=evb[:mb_max, :nt],
                        )
                    t0 += nt
    nc.compile()
    return nc


def kernel(x, kernels):
    from concourse.bass_utils import run_bass_kernel_spmd

    xt, wt, keep, Ms, C, U, T_out, nbins = _host_prep(x, kernels)
    B = xt.shape[0]
    assert B % N_CORES == 0
    b_per = B // N_CORES

    key = (b_per, C, U, T_out, tuple(keep), tuple(Ms))
    if key not in _prog_cache:
        _prog_cache[key] = _build_program(b_per, C, U, T_out, keep, Ms)
    nc = _prog_cache[key]

    in_maps = [
        {"xt": xt[c * b_per:(c + 1) * b_per], "wt": wt} for c in range(N_CORES)
    ]
    res = run_bass_kernel_spmd(nc, in_maps, list(range(N_CORES)))
    parts = [res.results[c]["out"] for c in range(N_CORES)]
    out = np.concatenate(parts, axis=0)  # (B, C, T_out)
    return np.ascontiguousarray(
        out.reshape(B, nbins, 2, T_out).transpose(0, 2, 1, 3)
    )
